# revision 15
# baseline (speedup 1.0000x reference)
"""CPC loss kernel for Trainium2, batch-sharded across 8 NeuronCores.

Shapes (hardcoded per problem spec):
  z, c: [2048, 64, 128] f32;  mask, neg_map: [128, 64] int;  W: [128, 128] f32
  ln_weight/ln_bias: [128] f32.  Output: scalar f32.

Per-core plan (Bc = 8 batch elements), bf16 data path:
  - Host packs per-core row tables: the pos/neg z gathers (with collided
    negatives zeroed, reproducing mask_from_map) land as zg [128L, 16seg*128]
    bf16, the c gather lands pre-transposed as cgt [128c, 8b*128L] bf16, and
    wpk = [W'^T | I].  Device reads only ~832KB contiguous per core.
  - LN stats via per-segment bn_stats (DVE); even/odd recombine and the
    Quake-rsqrt (2 Newton iters, eps and the /128 folded into the magic
    constants) run on the otherwise-idle gpsimd, so ACT only ever needs the
    exp_and_others table set (one load, prefetched by a dummy at t~0).
  - Normalize with the GroupNorm idiom: one tensor_scalar per segment with
    per-partition (mu, rstd) vector scalars, bf16 out.
  - PE transposes zln; MT computed as pm2[i,j] = zt^T E per batch half, so
    den = sum_i exp and num = diag(exp) are accumulating PE matmuls against
    a ones column (identity-masked product for the diagonal, on gpsimd).
  - Device outputs num/den [128, 16] f32; host does log(num/den + 1e-3) and
    the mean in float64.

ln_weight folds into W on the host; ln_bias cancels in the softmax.  No
max-subtraction needed: |logits| < ~70.
"""

import numpy as np

SEQ, B, L, ZD, CD = 2048, 64, 128, 128, 128
NCORES = 8
BC = B // NCORES  # 8
NSEG = 2 * BC  # 16 LN segments per core (interleaved pos/neg)
LN_EPS = 1e-5
SEM_STOP = 168  # min workable; exit sem sweep is ~65ns/sem

_cached = None


def _build_program():
    import concourse.bacc as bacc
    import concourse.tile as tile
    from concourse import bass as _bass
    from concourse import mybir

    # Fewer kernel semaphores -> shorter fixed entry/exit semaphore sweeps.
    orig_range = _bass.get_kernel_semaphore_range
    _bass.get_kernel_semaphore_range = lambda: range(
        orig_range().start, SEM_STOP
    )

    f32 = mybir.dt.float32
    bf16 = mybir.dt.bfloat16
    i32 = mybir.dt.int32
    AF = mybir.ActivationFunctionType
    ALU = mybir.AluOpType

    try:
        nc = bacc.Bacc(
            "TRN2",
            target_bir_lowering=False,
            debug=False,
            enable_asserts=True,
            num_devices=NCORES,
        )

        zg0_d = nc.dram_tensor("zg0", [128, 8 * ZD], bf16, kind="ExternalInput")
        zg1_d = nc.dram_tensor("zg1", [128, 8 * ZD], bf16, kind="ExternalInput")
        cgt_d = nc.dram_tensor("cgt", [128, BC * L], bf16, kind="ExternalInput")
        wpk_d = nc.dram_tensor("wpk", [128, 256], bf16, kind="ExternalInput")
        out_d = nc.dram_tensor("out", [128, NSEG], f32, kind="ExternalOutput")

        with tile.TileContext(nc) as tc:
            with (
                tc.tile_pool(name="singles", bufs=1) as singles,
                tc.tile_pool(name="szt", bufs=3) as szt,
                tc.tile_pool(name="sexp", bufs=2) as sexp,
                tc.tile_pool(name="ppe", bufs=2, space="PSUM") as ppe,
                tc.tile_pool(name="ppzt", bufs=2, space="PSUM") as ppzt,
                tc.tile_pool(name="ppmt", bufs=2, space="PSUM") as ppmt,
                tc.tile_pool(name="ppout", bufs=1, space="PSUM") as ppout,
            ):
                # ---- ACT table preloads: natural_log then exp_and_others,
                # both fetched by dummies before the input DMAs land.
                junk = singles.tile([128, 1], f32)
                nc.vector.memset(junk[:], 1.0)
                nc.scalar.activation(junk[:], junk[:], AF.Ln)
                nc.scalar.activation(junk[:], junk[:], AF.Exp)

                # ---- input DMAs: zg quartered so stats start early ----
                zg = singles.tile([128, NSEG * ZD], bf16)
                for q in range(4):
                    src = (zg0_d, zg1_d)[q // 2]
                    half = (q % 2) * 512
                    nc.sync.dma_start(
                        out=zg[:, q * 512 : (q + 1) * 512],
                        in_=src.ap()[:, half : half + 512],
                    )
                wpk = singles.tile([128, 256], bf16)
                nc.sync.dma_start(wpk[:], wpk_d.ap())
                cgt = singles.tile([128, BC * L], bf16)
                nc.sync.dma_start(cgt[:], cgt_d.ap())
                wt = wpk[:, 0:128]
                identb = wpk[:, 128:256]

                # ---- c-side: E[z, b*L+j] = sum_c W'[z,c] c_t[j,b,c] ----
                e_sb = singles.tile([128, BC * L], bf16)
                for g in range(2):
                    pe = ppe.tile([128, 512], f32, tag="pe")
                    nc.tensor.matmul(
                        out=pe[:],
                        lhsT=wt,
                        rhs=cgt[:, g * 512 : (g + 1) * 512],
                        start=True,
                        stop=True,
                    )
                    nc.scalar.copy(e_sb[:, g * 512 : (g + 1) * 512], pe[:])

                # ---- LN stats: bn_stats per segment (HW: out = 6/part) ----
                st = singles.tile([128, NSEG, 6], f32)
                for s in range(NSEG):
                    nc.vector.bn_stats(
                        out=st[:, s, :], in_=zg[:, s * ZD : (s + 1) * ZD]
                    )

                # ---- even/odd recombine on DVE (it idles here anyway);
                # rstd = exp(-0.5*ln(128*(var+eps)/128)) on ACT, one op each.
                # bn_stats gives (cnt, mean, cnt*var) of even and odd elems.
                # mu = (me+mo)/2;  128*(var+eps) = (sve+svo) + 32*d^2 + 128eps
                mu = singles.tile([128, NSEG], f32)
                dt_ = singles.tile([128, NSEG], f32)
                dd = singles.tile([128, NSEG], f32)
                vt = singles.tile([128, NSEG], f32)
                sv = singles.tile([128, NSEG], f32)
                vvx = singles.tile([128, NSEG], f32)
                lnv = singles.tile([128, NSEG], f32)
                rstd = singles.tile([128, NSEG], f32)

                def stv(k):
                    return st[:, :, k : k + 1]

                u = lambda t: t[:].unsqueeze(-1)
                nc.vector.tensor_tensor(
                    out=u(dt_), in0=stv(1), in1=stv(4), op=ALU.subtract
                )
                nc.vector.tensor_tensor(
                    out=u(dd), in0=u(dt_), in1=u(dt_), op=ALU.mult
                )
                nc.vector.tensor_scalar(
                    out=dd[:], in0=dd[:], scalar1=32.0,
                    scalar2=128.0 * LN_EPS, op0=ALU.mult, op1=ALU.add,
                )
                nc.vector.tensor_tensor(
                    out=u(vt), in0=stv(2), in1=stv(5), op=ALU.add
                )
                nc.vector.tensor_tensor(
                    out=vvx[:], in0=vt[:], in1=dd[:], op=ALU.add
                )
                nc.vector.tensor_tensor(
                    out=u(sv), in0=stv(1), in1=stv(4), op=ALU.add
                )
                nc.vector.tensor_scalar(
                    out=mu[:], in0=sv[:], scalar1=0.5, scalar2=None,
                    op0=ALU.mult,
                )
                nc.scalar.activation(lnv[:], vvx[:], AF.Ln, scale=1.0 / 128.0)
                nc.scalar.activation(rstd[:], lnv[:], AF.Exp, scale=-0.5)

                # ---- normalize: one tensor_scalar per segment ----
                zln = singles.tile([128, NSEG * ZD], bf16)
                for s in range(NSEG):
                    nc.vector.tensor_scalar(
                        out=zln[:, s * ZD : (s + 1) * ZD],
                        in0=zg[:, s * ZD : (s + 1) * ZD],
                        scalar1=mu[:, s : s + 1],
                        scalar2=rstd[:, s : s + 1],
                        op0=ALU.subtract,
                        op1=ALU.mult,
                    )

                # ---- per pair p: transposes; MT in [i-part, j-free] layout;
                # den and num(diag) as accumulating PE matmuls vs ones col.
                outv = singles.tile([128, NSEG], f32)  # [num | den]
                onescol = singles.tile([128, 1], bf16)
                nc.vector.memset(onescol[:], 1.0)
                outp = ppout.tile([128, NSEG], f32, tag="outp")
                for p in range(4):
                    pzt = ppzt.tile([128, 512], bf16, tag="pzt")
                    for k in range(4):
                        s = 4 * p + k
                        nc.tensor.transpose(
                            out=pzt[:, k * 128 : (k + 1) * 128],
                            in_=zln[:, s * ZD : (s + 1) * ZD],
                            identity=identb,
                        )
                    zt = szt.tile([128, 512], bf16, tag="zt")
                    if p % 2 == 0:
                        nc.vector.tensor_copy(zt[:], pzt[:])
                    else:
                        nc.scalar.copy(zt[:], pzt[:])
                    # pm2 cols [(2k+h)*128]: batch 2p+k, half h (pos/neg i)
                    pm2 = ppmt.tile([128, 512], f32, tag="pm2")
                    for q in range(4):
                        b = 2 * p + q // 2
                        nc.tensor.matmul(
                            out=pm2[:, q * 128 : (q + 1) * 128],
                            lhsT=zt[:, q * 128 : (q + 1) * 128],
                            rhs=e_sb[:, b * L : (b + 1) * L],
                            start=True,
                            stop=True,
                        )
                    expm = sexp.tile([128, 512], bf16, tag="expm")
                    nc.scalar.activation(expm[:], pm2[:], AF.Exp)
                    for k in range(2):
                        b = 2 * p + k
                        # den[j,b] = sum_i expm[i, j]: accumulate both halves
                        nc.tensor.matmul(
                            out=outp[:, BC + b : BC + b + 1],
                            lhsT=expm[:, 2 * k * 128 : (2 * k + 1) * 128],
                            rhs=onescol[:],
                            start=True,
                            stop=False,
                        )
                        nc.tensor.matmul(
                            out=outp[:, BC + b : BC + b + 1],
                            lhsT=expm[:, (2 * k + 1) * 128 : (2 * k + 2) * 128],
                            rhs=onescol[:],
                            start=False,
                            stop=True,
                        )
                        # num[j,b] = expm[j,j] of the pos half (identity mask)
                        # gpsimd for early pairs; DVE (free by then, lower
                        # latency) for the last pair's output tail.
                        prod = szt.tile([128, 128], bf16, tag="prod")
                        eng = nc.vector if p == 3 else nc.gpsimd
                        eng.tensor_tensor(
                            out=prod[:],
                            in0=expm[:, 2 * k * 128 : (2 * k + 1) * 128],
                            in1=identb,
                            op=ALU.mult,
                        )
                        nc.tensor.matmul(
                            out=outp[:, b : b + 1],
                            lhsT=prod[:],
                            rhs=onescol[:],
                            start=True,
                            stop=True,
                        )
                nc.vector.tensor_copy(outv[:], outp[:])
                nc.sync.dma_start(out_d.ap(), outv[:])

        nc.compile()
        return nc
    finally:
        _bass.get_kernel_semaphore_range = orig_range


def _prep_in_maps(z, c, mask, neg_map, W, ln_weight):
    import ml_dtypes

    bf = ml_dtypes.bfloat16
    z = np.asarray(z, dtype=np.float32)
    c = np.asarray(c, dtype=np.float32)
    mask = np.asarray(mask).astype(np.int64)
    neg_map = np.asarray(neg_map).astype(np.int64)
    W = np.asarray(W, dtype=np.float32)
    ln_weight = np.asarray(ln_weight, dtype=np.float32)

    wt = (ln_weight[:, None] * W).T  # wt[c, z] = W'[z, c]
    wpk = np.ascontiguousarray(
        np.concatenate([wt, np.eye(128, dtype=np.float32)], axis=1)
    ).astype(bf)
    boff = np.arange(BC)[None, :]
    in_maps = []
    for i in range(NCORES):
        bsl = slice(i * BC, (i + 1) * BC)
        m = mask[:, bsl]  # [L, BC]
        n = neg_map[:, bsl]
        zb = z[:, bsl, :]
        cb = c[:, bsl, :]
        zpos = zb[m, boff, :]  # [L, BC, ZD]
        zneg = zb[n, boff, :]
        hit = (n[:, None, :] == m[None, :, :]).any(axis=1)  # [L, BC]
        zneg = np.where(hit[:, :, None], np.float32(0.0), zneg)
        zga = np.empty((L, NSEG, ZD), dtype=np.float32)
        zga[:, 0::2, :] = zpos
        zga[:, 1::2, :] = zneg
        zga16 = zga.astype(bf)
        zg0 = np.ascontiguousarray(zga16[:, 0:8, :].reshape(L, 8 * ZD))
        zg1 = np.ascontiguousarray(zga16[:, 8:16, :].reshape(L, 8 * ZD))
        cpos = cb[m, boff, :]  # [L(j), BC, CD]
        cgt = np.ascontiguousarray(
            cpos.transpose(2, 1, 0).reshape(CD, BC * L)
        ).astype(bf)
        in_maps.append({"zg0": zg0, "zg1": zg1, "cgt": cgt, "wpk": wpk})
    return in_maps


def _combine(results):
    total = np.float64(0.0)
    for r in results:
        o = np.asarray(r["out"], dtype=np.float64)
        num, den = o[:, 0:BC], o[:, BC : 2 * BC]
        total += np.log(num / den + 1e-3).sum()
    return np.float32(-(total / (L * B)))


def kernel(z, c, mask, neg_map, W, ln_weight, ln_bias):
    from concourse import bass_utils

    global _cached
    if _cached is None:
        _cached = _build_program()
    nc = _cached

    in_maps = _prep_in_maps(z, c, mask, neg_map, W, ln_weight)
    res = bass_utils.run_bass_kernel_spmd(
        nc, in_maps, core_ids=list(range(NCORES))
    )
    return _combine(res.results)


# revision 19
# speedup vs baseline: 1.1047x; 1.1047x over previous
"""CPC loss kernel for Trainium2, batch-sharded across 8 NeuronCores.

Shapes (hardcoded per problem spec):
  z, c: [2048, 64, 128] f32;  mask, neg_map: [128, 64] int;  W: [128, 128] f32
  ln_weight/ln_bias: [128] f32.  Output: scalar f32.

Per-core plan (Bc = 8 batch elements), bf16 data path:
  - Host packs per-core row tables: the pos/neg z gathers (with collided
    negatives zeroed, reproducing mask_from_map) land as zg [128L, 16seg*128]
    bf16, the c gather lands pre-transposed as cgt [128c, 8b*128L] bf16, and
    wpk = [W'^T | I].  Device reads only ~832KB contiguous per core.
  - LN stats via per-segment bn_stats (DVE); even/odd recombine and the
    Quake-rsqrt (2 Newton iters, eps and the /128 folded into the magic
    constants) run on the otherwise-idle gpsimd, so ACT only ever needs the
    exp_and_others table set (one load, prefetched by a dummy at t~0).
  - Normalize with the GroupNorm idiom: one tensor_scalar per segment with
    per-partition (mu, rstd) vector scalars, bf16 out.
  - PE transposes zln; MT computed as pm2[i,j] = zt^T E per batch half, so
    den = sum_i exp and num = diag(exp) are accumulating PE matmuls against
    a ones column (identity-masked product for the diagonal, on gpsimd).
  - Device outputs num/den [128, 16] f32; host does log(num/den + 1e-3) and
    the mean in float64.

ln_weight folds into W on the host; ln_bias cancels in the softmax.  No
max-subtraction needed: |logits| < ~70.
"""

import numpy as np

SEQ, B, L, ZD, CD = 2048, 64, 128, 128, 128
NCORES = 8
BC = B // NCORES  # 8
NSEG = 2 * BC  # 16 LN segments per core (interleaved pos/neg)
LN_EPS = 1e-5
SEM_STOP = 168  # min workable; exit sem sweep is ~65ns/sem

_cached = None


def _build_program():
    import concourse.bacc as bacc
    import concourse.tile as tile
    from concourse import bass as _bass
    from concourse import mybir

    # Fewer kernel semaphores -> shorter fixed entry/exit semaphore sweeps.
    orig_range = _bass.get_kernel_semaphore_range
    _bass.get_kernel_semaphore_range = lambda: range(
        orig_range().start, SEM_STOP
    )

    f32 = mybir.dt.float32
    bf16 = mybir.dt.bfloat16
    i32 = mybir.dt.int32
    AF = mybir.ActivationFunctionType
    ALU = mybir.AluOpType

    try:
        nc = bacc.Bacc(
            "TRN2",
            target_bir_lowering=False,
            debug=False,
            enable_asserts=True,
            num_devices=NCORES,
        )

        zg0_d = nc.dram_tensor("zg0", [128, 8 * ZD], bf16, kind="ExternalInput")
        zg1_d = nc.dram_tensor("zg1", [128, 8 * ZD], bf16, kind="ExternalInput")
        cgt_d = nc.dram_tensor("cgt", [128, BC * L], bf16, kind="ExternalInput")
        wpk_d = nc.dram_tensor("wpk", [128, 256], bf16, kind="ExternalInput")
        out_d = nc.dram_tensor("out", [128, NSEG], f32, kind="ExternalOutput")

        with tile.TileContext(nc) as tc:
            with (
                tc.tile_pool(name="singles", bufs=1) as singles,
                tc.tile_pool(name="szt", bufs=3) as szt,
                tc.tile_pool(name="sexp", bufs=2) as sexp,
                tc.tile_pool(name="ppe", bufs=2, space="PSUM") as ppe,
                tc.tile_pool(name="ppzt", bufs=2, space="PSUM") as ppzt,
                tc.tile_pool(name="ppmt", bufs=2, space="PSUM") as ppmt,
                tc.tile_pool(name="ppout", bufs=1, space="PSUM") as ppout,
            ):
                # ---- ACT table preload: exp_and_others is the ONLY set the
                # kernel ever needs (Exp + Copy); fetched before data lands.
                junk = singles.tile([128, 1], f32)
                nc.vector.memset(junk[:], 1.0)
                nc.scalar.activation(junk[:], junk[:], AF.Exp)

                # ---- input DMAs: zg quartered so stats start early ----
                zg = singles.tile([128, NSEG * ZD], bf16)
                for q in range(4):
                    src = (zg0_d, zg1_d)[q // 2]
                    half = (q % 2) * 512
                    nc.sync.dma_start(
                        out=zg[:, q * 512 : (q + 1) * 512],
                        in_=src.ap()[:, half : half + 512],
                    )
                wpk = singles.tile([128, 256], bf16)
                nc.sync.dma_start(wpk[:], wpk_d.ap())
                cgt = singles.tile([128, BC * L], bf16)
                nc.sync.dma_start(cgt[:], cgt_d.ap())
                wt = wpk[:, 0:128]
                identb = wpk[:, 128:256]

                # ---- c-side: E[z, b*L+j] = sum_c W'[z,c] c_t[j,b,c] ----
                e_sb = singles.tile([128, BC * L], bf16)
                for g in range(2):
                    pe = ppe.tile([128, 512], f32, tag="pe")
                    nc.tensor.matmul(
                        out=pe[:],
                        lhsT=wt,
                        rhs=cgt[:, g * 512 : (g + 1) * 512],
                        start=True,
                        stop=True,
                    )
                    nc.scalar.copy(e_sb[:, g * 512 : (g + 1) * 512], pe[:])

                # ---- LN stats: bn_stats per segment (HW: out = 6/part) ----
                st = singles.tile([128, NSEG, 6], f32)
                for s in range(NSEG):
                    nc.vector.bn_stats(
                        out=st[:, s, :], in_=zg[:, s * ZD : (s + 1) * ZD]
                    )

                # ---- even/odd recombine on DVE (it idles here anyway);
                # rstd = exp(-0.5*ln(128*(var+eps)/128)) on ACT, one op each.
                # bn_stats gives (cnt, mean, cnt*var) of even and odd elems.
                # mu = (me+mo)/2;  128*(var+eps) = (sve+svo) + 32*d^2 + 128eps
                mu = singles.tile([128, NSEG], f32)
                dt_ = singles.tile([128, NSEG], f32)
                dd = singles.tile([128, NSEG], f32)
                vt = singles.tile([128, NSEG], f32)
                sv = singles.tile([128, NSEG], f32)
                vvx = singles.tile([128, NSEG], f32)
                lnv = singles.tile([128, NSEG], f32)
                rstd = singles.tile([128, NSEG], f32)

                def stv(k):
                    return st[:, :, k : k + 1]

                u = lambda t: t[:].unsqueeze(-1)
                nc.vector.tensor_tensor(
                    out=u(dt_), in0=stv(1), in1=stv(4), op=ALU.subtract
                )
                nc.vector.tensor_tensor(
                    out=u(dd), in0=u(dt_), in1=u(dt_), op=ALU.mult
                )
                nc.vector.tensor_scalar(
                    out=dd[:], in0=dd[:], scalar1=32.0,
                    scalar2=128.0 * LN_EPS, op0=ALU.mult, op1=ALU.add,
                )
                nc.vector.tensor_tensor(
                    out=u(vt), in0=stv(2), in1=stv(5), op=ALU.add
                )
                nc.vector.tensor_tensor(
                    out=vvx[:], in0=vt[:], in1=dd[:], op=ALU.add
                )
                nc.vector.tensor_tensor(
                    out=u(sv), in0=stv(1), in1=stv(4), op=ALU.add
                )
                nc.vector.tensor_scalar(
                    out=mu[:], in0=sv[:], scalar1=0.5, scalar2=None,
                    op0=ALU.mult,
                )
                # Quake rsqrt of vv = vvx/128 (the /128 folds into the magic
                # constant and the Newton -0.5 scale); one Newton iteration
                # brings the ~3e-3 seed error to ~2e-5.
                nc.vector.tensor_scalar(
                    out=rstd[:].bitcast(i32), in0=vvx[:].bitcast(i32),
                    scalar1=1, scalar2=None, op0=ALU.arith_shift_right,
                )
                nc.vector.tensor_scalar(
                    out=rstd[:].bitcast(i32), in0=rstd[:].bitcast(i32),
                    scalar1=-1, scalar2=0x5F3759DF + 0x01C00000,
                    op0=ALU.mult, op1=ALU.add,
                )
                nc.vector.tensor_tensor(
                    out=lnv[:], in0=rstd[:], in1=rstd[:], op=ALU.mult
                )
                nc.vector.tensor_tensor(
                    out=lnv[:], in0=lnv[:], in1=vvx[:], op=ALU.mult
                )
                nc.vector.tensor_scalar(
                    out=lnv[:], in0=lnv[:], scalar1=-0.5 / 128.0, scalar2=1.5,
                    op0=ALU.mult, op1=ALU.add,
                )
                nc.vector.tensor_tensor(
                    out=rstd[:], in0=rstd[:], in1=lnv[:], op=ALU.mult
                )

                # ---- normalize: one tensor_scalar per segment ----
                zln = singles.tile([128, NSEG * ZD], bf16)
                for s in range(NSEG):
                    nc.vector.tensor_scalar(
                        out=zln[:, s * ZD : (s + 1) * ZD],
                        in0=zg[:, s * ZD : (s + 1) * ZD],
                        scalar1=mu[:, s : s + 1],
                        scalar2=rstd[:, s : s + 1],
                        op0=ALU.subtract,
                        op1=ALU.mult,
                    )

                # ---- per pair p: transposes; MT in [i-part, j-free] layout;
                # den and num(diag) as accumulating PE matmuls vs ones col.
                outv = singles.tile([128, NSEG], f32)  # [num | den]
                onescol = singles.tile([128, 1], bf16)
                nc.vector.memset(onescol[:], 1.0)
                outp = ppout.tile([128, NSEG], f32, tag="outp")
                for p in range(4):
                    pzt = ppzt.tile([128, 512], bf16, tag="pzt")
                    for k in range(4):
                        s = 4 * p + k
                        nc.tensor.transpose(
                            out=pzt[:, k * 128 : (k + 1) * 128],
                            in_=zln[:, s * ZD : (s + 1) * ZD],
                            identity=identb,
                        )
                    zt = szt.tile([128, 512], bf16, tag="zt")
                    if p % 2 == 0:
                        nc.vector.tensor_copy(zt[:], pzt[:])
                    else:
                        nc.scalar.copy(zt[:], pzt[:])
                    # pm2 cols [(2k+h)*128]: batch 2p+k, half h (pos/neg i)
                    pm2 = ppmt.tile([128, 512], f32, tag="pm2")
                    for q in range(4):
                        b = 2 * p + q // 2
                        nc.tensor.matmul(
                            out=pm2[:, q * 128 : (q + 1) * 128],
                            lhsT=zt[:, q * 128 : (q + 1) * 128],
                            rhs=e_sb[:, b * L : (b + 1) * L],
                            start=True,
                            stop=True,
                        )
                    expm = sexp.tile([128, 512], bf16, tag="expm")
                    nc.scalar.activation(expm[:], pm2[:], AF.Exp)
                    for k in range(2):
                        b = 2 * p + k
                        # den[j,b] = sum_i expm[i, j]: accumulate both halves
                        nc.tensor.matmul(
                            out=outp[:, BC + b : BC + b + 1],
                            lhsT=expm[:, 2 * k * 128 : (2 * k + 1) * 128],
                            rhs=onescol[:],
                            start=True,
                            stop=False,
                        )
                        nc.tensor.matmul(
                            out=outp[:, BC + b : BC + b + 1],
                            lhsT=expm[:, (2 * k + 1) * 128 : (2 * k + 2) * 128],
                            rhs=onescol[:],
                            start=False,
                            stop=True,
                        )
                        # num[j,b] = expm[j,j] of the pos half (identity mask)
                        # gpsimd for early pairs; DVE (free by then, lower
                        # latency) for the last pair's output tail.
                        prod = szt.tile([128, 128], bf16, tag="prod")
                        eng = nc.vector if p == 3 else nc.gpsimd
                        eng.tensor_tensor(
                            out=prod[:],
                            in0=expm[:, 2 * k * 128 : (2 * k + 1) * 128],
                            in1=identb,
                            op=ALU.mult,
                        )
                        nc.tensor.matmul(
                            out=outp[:, b : b + 1],
                            lhsT=prod[:],
                            rhs=onescol[:],
                            start=True,
                            stop=True,
                        )
                nc.vector.tensor_copy(outv[:], outp[:])
                nc.sync.dma_start(out_d.ap(), outv[:])

        nc.compile()
        return nc
    finally:
        _bass.get_kernel_semaphore_range = orig_range


def _prep_in_maps(z, c, mask, neg_map, W, ln_weight):
    import ml_dtypes

    bf = ml_dtypes.bfloat16
    z = np.asarray(z, dtype=np.float32)
    c = np.asarray(c, dtype=np.float32)
    mask = np.asarray(mask).astype(np.int64)
    neg_map = np.asarray(neg_map).astype(np.int64)
    W = np.asarray(W, dtype=np.float32)
    ln_weight = np.asarray(ln_weight, dtype=np.float32)

    wt = (ln_weight[:, None] * W).T  # wt[c, z] = W'[z, c]
    wpk = np.ascontiguousarray(
        np.concatenate([wt, np.eye(128, dtype=np.float32)], axis=1)
    ).astype(bf)
    boff = np.arange(BC)[None, :]
    in_maps = []
    for i in range(NCORES):
        bsl = slice(i * BC, (i + 1) * BC)
        m = mask[:, bsl]  # [L, BC]
        n = neg_map[:, bsl]
        zb = z[:, bsl, :]
        cb = c[:, bsl, :]
        zpos = zb[m, boff, :]  # [L, BC, ZD]
        zneg = zb[n, boff, :]
        hit = (n[:, None, :] == m[None, :, :]).any(axis=1)  # [L, BC]
        zneg = np.where(hit[:, :, None], np.float32(0.0), zneg)
        zga = np.empty((L, NSEG, ZD), dtype=np.float32)
        zga[:, 0::2, :] = zpos
        zga[:, 1::2, :] = zneg
        zga16 = zga.astype(bf)
        zg0 = np.ascontiguousarray(zga16[:, 0:8, :].reshape(L, 8 * ZD))
        zg1 = np.ascontiguousarray(zga16[:, 8:16, :].reshape(L, 8 * ZD))
        cpos = cb[m, boff, :]  # [L(j), BC, CD]
        cgt = np.ascontiguousarray(
            cpos.transpose(2, 1, 0).reshape(CD, BC * L)
        ).astype(bf)
        in_maps.append({"zg0": zg0, "zg1": zg1, "cgt": cgt, "wpk": wpk})
    return in_maps


def _combine(results):
    total = np.float64(0.0)
    for r in results:
        o = np.asarray(r["out"], dtype=np.float64)
        num, den = o[:, 0:BC], o[:, BC : 2 * BC]
        total += np.log(num / den + 1e-3).sum()
    return np.float32(-(total / (L * B)))


def kernel(z, c, mask, neg_map, W, ln_weight, ln_bias):
    from concourse import bass_utils

    global _cached
    if _cached is None:
        _cached = _build_program()
    nc = _cached

    in_maps = _prep_in_maps(z, c, mask, neg_map, W, ln_weight)
    res = bass_utils.run_bass_kernel_spmd(
        nc, in_maps, core_ids=list(range(NCORES))
    )
    return _combine(res.results)


# revision 25
# speedup vs baseline: 1.1075x; 1.0025x over previous
"""CPC loss kernel for Trainium2, batch-sharded across 8 NeuronCores.

Shapes (hardcoded per problem spec):
  z, c: [2048, 64, 128] f32;  mask, neg_map: [128, 64] int;  W: [128, 128] f32
  ln_weight/ln_bias: [128] f32.  Output: scalar f32.

Per-core plan (Bc = 8 batch elements), bf16 data path:
  - Host packs per-core row tables: the pos/neg z gathers (with collided
    negatives zeroed, reproducing mask_from_map) land as zg [128L, 16seg*128]
    bf16, the c gather lands pre-transposed as cgt [128c, 8b*128L] bf16, and
    wpk = [W'^T | I].  Device reads only ~832KB contiguous per core.
  - LN stats via per-segment bn_stats (DVE); even/odd recombine and the
    Quake-rsqrt (2 Newton iters, eps and the /128 folded into the magic
    constants) run on the otherwise-idle gpsimd, so ACT only ever needs the
    exp_and_others table set (one load, prefetched by a dummy at t~0).
  - Normalize with the GroupNorm idiom: one tensor_scalar per segment with
    per-partition (mu, rstd) vector scalars, bf16 out.
  - PE transposes zln; MT computed as pm2[i,j] = zt^T E per batch half, so
    den = sum_i exp and num = diag(exp) are accumulating PE matmuls against
    a ones column (identity-masked product for the diagonal, on gpsimd).
  - Device outputs num/den [128, 16] f32; host does log(num/den + 1e-3) and
    the mean in float64.

ln_weight folds into W on the host; ln_bias cancels in the softmax.  No
max-subtraction needed: |logits| < ~70.
"""

import numpy as np

SEQ, B, L, ZD, CD = 2048, 64, 128, 128, 128
NCORES = 8
BC = B // NCORES  # 8
NSEG = 2 * BC  # 16 LN segments per core (interleaved pos/neg)
LN_EPS = 1e-5
SEM_STOP = 168  # min workable; exit sem sweep is ~65ns/sem

_cached = None


def _build_program():
    import concourse.bacc as bacc
    import concourse.tile as tile
    from concourse import bass as _bass
    from concourse import mybir

    # Fewer kernel semaphores -> shorter fixed entry/exit semaphore sweeps.
    orig_range = _bass.get_kernel_semaphore_range
    _bass.get_kernel_semaphore_range = lambda: range(
        orig_range().start, SEM_STOP
    )

    f32 = mybir.dt.float32
    bf16 = mybir.dt.bfloat16
    i32 = mybir.dt.int32
    AF = mybir.ActivationFunctionType
    ALU = mybir.AluOpType

    try:
        nc = bacc.Bacc(
            "TRN2",
            target_bir_lowering=False,
            debug=False,
            enable_asserts=True,
            num_devices=NCORES,
        )

        zg0_d = nc.dram_tensor("zg0", [128, 8 * ZD], bf16, kind="ExternalInput")
        zg1_d = nc.dram_tensor("zg1", [128, 8 * ZD], bf16, kind="ExternalInput")
        cgt_d = nc.dram_tensor("cgt", [128, BC * L], bf16, kind="ExternalInput")
        wpk_d = nc.dram_tensor("wpk", [128, 256], bf16, kind="ExternalInput")
        out_d = nc.dram_tensor("out", [NSEG, 128], f32, kind="ExternalOutput")

        with tile.TileContext(nc) as tc:
            with (
                tc.tile_pool(name="singles", bufs=1) as singles,
                tc.tile_pool(name="szt", bufs=3) as szt,
                tc.tile_pool(name="sexp", bufs=2) as sexp,
                tc.tile_pool(name="ppe", bufs=2, space="PSUM") as ppe,
                tc.tile_pool(name="ppzt", bufs=2, space="PSUM") as ppzt,
                tc.tile_pool(name="ppmt", bufs=2, space="PSUM") as ppmt,
                tc.tile_pool(name="ppout", bufs=1, space="PSUM") as ppout,
            ):
                # ---- ACT table preload: exp_and_others is the ONLY set the
                # kernel ever needs (Exp + Copy); fetched before data lands.
                junk = singles.tile([128, 1], f32)
                nc.vector.memset(junk[:], 1.0)
                nc.scalar.activation(junk[:], junk[:], AF.Exp)

                # ---- input DMAs: zg quartered so stats start early ----
                zg = singles.tile([128, NSEG * ZD], bf16)
                for q in range(4):
                    src = (zg0_d, zg1_d)[q // 2]
                    half = (q % 2) * 512
                    nc.sync.dma_start(
                        out=zg[:, q * 512 : (q + 1) * 512],
                        in_=src.ap()[:, half : half + 512],
                    )
                wpk = singles.tile([128, 256], bf16)
                nc.sync.dma_start(wpk[:], wpk_d.ap())
                cgt = singles.tile([128, BC * L], bf16)
                nc.sync.dma_start(cgt[:], cgt_d.ap())
                wt = wpk[:, 0:128]
                identb = wpk[:, 128:256]

                # ---- c-side: E[z, b*L+j] = sum_c W'[z,c] c_t[j,b,c] ----
                e_sb = singles.tile([128, BC * L], bf16)
                for g in range(2):
                    pe = ppe.tile([128, 512], f32, tag="pe")
                    nc.tensor.matmul(
                        out=pe[:],
                        lhsT=wt,
                        rhs=cgt[:, g * 512 : (g + 1) * 512],
                        start=True,
                        stop=True,
                    )
                    nc.scalar.copy(e_sb[:, g * 512 : (g + 1) * 512], pe[:])

                # ---- LN stats: bn_stats per segment (HW: out = 6/part) ----
                st = singles.tile([128, NSEG, 6], f32)
                for s in range(NSEG):
                    nc.vector.bn_stats(
                        out=st[:, s, :], in_=zg[:, s * ZD : (s + 1) * ZD]
                    )

                # ---- even/odd recombine on DVE (it idles here anyway);
                # rstd = exp(-0.5*ln(128*(var+eps)/128)) on ACT, one op each.
                # bn_stats gives (cnt, mean, cnt*var) of even and odd elems.
                # mu = (me+mo)/2;  128*(var+eps) = (sve+svo) + 32*d^2 + 128eps
                mu = singles.tile([128, NSEG], f32)
                dt_ = singles.tile([128, NSEG], f32)
                dd = singles.tile([128, NSEG], f32)
                vt = singles.tile([128, NSEG], f32)
                sv = singles.tile([128, NSEG], f32)
                vvx = singles.tile([128, NSEG], f32)
                lnv = singles.tile([128, NSEG], f32)
                rstd = singles.tile([128, NSEG], f32)

                def stv(k):
                    return st[:, :, k : k + 1]

                u = lambda t: t[:].unsqueeze(-1)
                nc.vector.tensor_tensor(
                    out=u(dt_), in0=stv(1), in1=stv(4), op=ALU.subtract
                )
                nc.vector.tensor_tensor(
                    out=u(dd), in0=u(dt_), in1=u(dt_), op=ALU.mult
                )
                nc.vector.tensor_scalar(
                    out=dd[:], in0=dd[:], scalar1=32.0,
                    scalar2=128.0 * LN_EPS, op0=ALU.mult, op1=ALU.add,
                )
                nc.vector.tensor_tensor(
                    out=u(vt), in0=stv(2), in1=stv(5), op=ALU.add
                )
                nc.vector.tensor_tensor(
                    out=vvx[:], in0=vt[:], in1=dd[:], op=ALU.add
                )
                nc.vector.tensor_tensor(
                    out=u(sv), in0=stv(1), in1=stv(4), op=ALU.add
                )
                nc.vector.tensor_scalar(
                    out=mu[:], in0=sv[:], scalar1=0.5, scalar2=None,
                    op0=ALU.mult,
                )
                # Quake rsqrt of vv = vvx/128 (the /128 folds into the magic
                # constant and the Newton -0.5 scale); one Newton iteration
                # brings the ~3e-3 seed error to ~2e-5.
                nc.vector.tensor_scalar(
                    out=rstd[:].bitcast(i32), in0=vvx[:].bitcast(i32),
                    scalar1=1, scalar2=None, op0=ALU.arith_shift_right,
                )
                nc.vector.tensor_scalar(
                    out=rstd[:].bitcast(i32), in0=rstd[:].bitcast(i32),
                    scalar1=-1, scalar2=0x5F3759DF + 0x01C00000,
                    op0=ALU.mult, op1=ALU.add,
                )
                nc.vector.tensor_tensor(
                    out=lnv[:], in0=rstd[:], in1=rstd[:], op=ALU.mult
                )
                nc.vector.tensor_tensor(
                    out=lnv[:], in0=lnv[:], in1=vvx[:], op=ALU.mult
                )
                nc.vector.tensor_scalar(
                    out=lnv[:], in0=lnv[:], scalar1=-0.5 / 128.0, scalar2=1.5,
                    op0=ALU.mult, op1=ALU.add,
                )
                nc.vector.tensor_tensor(
                    out=rstd[:], in0=rstd[:], in1=lnv[:], op=ALU.mult
                )

                # ---- normalize: one tensor_scalar per segment ----
                zln = singles.tile([128, NSEG * ZD], bf16)

                def norm_seg(s):
                    nc.vector.tensor_scalar(
                        out=zln[:, s * ZD : (s + 1) * ZD],
                        in0=zg[:, s * ZD : (s + 1) * ZD],
                        scalar1=mu[:, s : s + 1],
                        scalar2=rstd[:, s : s + 1],
                        op0=ALU.subtract,
                        op1=ALU.mult,
                    )

                for s in range(8):
                    norm_seg(s)

                # ---- per pair p: transposes; MT in [i-part, j-free] layout.
                # num/den land as [1,128] PSUM rows (lhsT=ones, rhs=expm) in
                # a [16,128] tile: rows 0-7 num, 8-15 den -> 16-desc out DMA.
                outv = singles.tile([128, NSEG], bf16)
                outvT = singles.tile([NSEG, 128], f32)
                onescol = singles.tile([128, 1], bf16)
                nc.vector.memset(onescol[:], 1.0)
                outp = ppout.tile([128, NSEG], f32, tag="outp")

                def pair(p):
                    pzt = ppzt.tile([128, 512], bf16, tag="pzt")
                    for k in range(4):
                        s = 4 * p + k
                        nc.tensor.transpose(
                            out=pzt[:, k * 128 : (k + 1) * 128],
                            in_=zln[:, s * ZD : (s + 1) * ZD],
                            identity=identb,
                        )
                    zt = szt.tile([128, 512], bf16, tag="zt")
                    if p % 2 == 0:
                        nc.vector.tensor_copy(zt[:], pzt[:])
                    else:
                        nc.scalar.copy(zt[:], pzt[:])
                    # pm2 cols [(2k+h)*128]: batch 2p+k, half h (pos/neg i)
                    pm2 = ppmt.tile([128, 512], f32, tag="pm2")
                    for q in range(4):
                        b = 2 * p + q // 2
                        nc.tensor.matmul(
                            out=pm2[:, q * 128 : (q + 1) * 128],
                            lhsT=zt[:, q * 128 : (q + 1) * 128],
                            rhs=e_sb[:, b * L : (b + 1) * L],
                            start=True,
                            stop=True,
                        )
                    expm = sexp.tile([128, 512], bf16, tag="expm")
                    nc.scalar.activation(expm[:], pm2[:], AF.Exp)
                    for k in range(2):
                        b = 2 * p + k
                        # den[j,b] = sum_i expm[i, j]: accumulate both halves
                        nc.tensor.matmul(
                            out=outp[:, BC + b : BC + b + 1],
                            lhsT=expm[:, 2 * k * 128 : (2 * k + 1) * 128],
                            rhs=onescol[:],
                            start=True,
                            stop=False,
                        )
                        nc.tensor.matmul(
                            out=outp[:, BC + b : BC + b + 1],
                            lhsT=expm[:, (2 * k + 1) * 128 : (2 * k + 2) * 128],
                            rhs=onescol[:],
                            start=False,
                            stop=True,
                        )
                        # num[b-row, j] = expm[j,j] of pos half (identity mask)
                        # gpsimd for early pairs; DVE (free by then, lower
                        # latency) for the last pair's output tail.
                        prod = szt.tile([128, 128], bf16, tag="prod")
                        eng = nc.vector if p == 3 else nc.gpsimd
                        eng.tensor_tensor(
                            out=prod[:],
                            in0=expm[:, 2 * k * 128 : (2 * k + 1) * 128],
                            in1=identb,
                            op=ALU.mult,
                        )
                        nc.tensor.matmul(
                            out=outp[:, b : b + 1],
                            lhsT=prod[:],
                            rhs=onescol[:],
                            start=True,
                            stop=True,
                        )

                pair(0)
                for s in range(8, NSEG):
                    norm_seg(s)
                for p in range(1, 4):
                    pair(p)
                # transpose [128,16]->[16,128] so the out DMA is 16 fat
                # descriptors instead of 128 64B ones (fast completion).
                nc.vector.tensor_copy(outv[:], outp[:])
                pot = ppout.tile([NSEG, 128], bf16, tag="pot")
                nc.tensor.transpose(out=pot[:], in_=outv[:], identity=identb)
                nc.vector.tensor_copy(outvT[:], pot[:])
                nc.sync.dma_start(out_d.ap(), outvT[:])

        nc.compile()
        return nc
    finally:
        _bass.get_kernel_semaphore_range = orig_range


def _prep_in_maps(z, c, mask, neg_map, W, ln_weight):
    import ml_dtypes

    bf = ml_dtypes.bfloat16
    z = np.asarray(z, dtype=np.float32)
    c = np.asarray(c, dtype=np.float32)
    mask = np.asarray(mask).astype(np.int64)
    neg_map = np.asarray(neg_map).astype(np.int64)
    W = np.asarray(W, dtype=np.float32)
    ln_weight = np.asarray(ln_weight, dtype=np.float32)

    wt = (ln_weight[:, None] * W).T  # wt[c, z] = W'[z, c]
    wpk = np.ascontiguousarray(
        np.concatenate([wt, np.eye(128, dtype=np.float32)], axis=1)
    ).astype(bf)
    boff = np.arange(BC)[None, :]
    in_maps = []
    for i in range(NCORES):
        bsl = slice(i * BC, (i + 1) * BC)
        m = mask[:, bsl]  # [L, BC]
        n = neg_map[:, bsl]
        zb = z[:, bsl, :]
        cb = c[:, bsl, :]
        zpos = zb[m, boff, :]  # [L, BC, ZD]
        zneg = zb[n, boff, :]
        hit = (n[:, None, :] == m[None, :, :]).any(axis=1)  # [L, BC]
        zneg = np.where(hit[:, :, None], np.float32(0.0), zneg)
        zga = np.empty((L, NSEG, ZD), dtype=np.float32)
        zga[:, 0::2, :] = zpos
        zga[:, 1::2, :] = zneg
        zga16 = zga.astype(bf)
        zg0 = np.ascontiguousarray(zga16[:, 0:8, :].reshape(L, 8 * ZD))
        zg1 = np.ascontiguousarray(zga16[:, 8:16, :].reshape(L, 8 * ZD))
        cpos = cb[m, boff, :]  # [L(j), BC, CD]
        cgt = np.ascontiguousarray(
            cpos.transpose(2, 1, 0).reshape(CD, BC * L)
        ).astype(bf)
        in_maps.append({"zg0": zg0, "zg1": zg1, "cgt": cgt, "wpk": wpk})
    return in_maps


def _combine(results):
    total = np.float64(0.0)
    for r in results:
        o = np.asarray(r["out"], dtype=np.float64)  # [16, 128]: num rows, den rows
        num, den = o[0:BC, :], o[BC : 2 * BC, :]
        total += np.log(num / den + 1e-3).sum()
    return np.float32(-(total / (L * B)))


def kernel(z, c, mask, neg_map, W, ln_weight, ln_bias):
    from concourse import bass_utils

    global _cached
    if _cached is None:
        _cached = _build_program()
    nc = _cached

    in_maps = _prep_in_maps(z, c, mask, neg_map, W, ln_weight)
    res = bass_utils.run_bass_kernel_spmd(
        nc, in_maps, core_ids=list(range(NCORES))
    )
    return _combine(res.results)


# revision 26
# speedup vs baseline: 1.1449x; 1.0338x over previous
"""CPC loss kernel for Trainium2, batch-sharded across 8 NeuronCores.

Shapes (hardcoded per problem spec):
  z, c: [2048, 64, 128] f32;  mask, neg_map: [128, 64] int;  W: [128, 128] f32
  ln_weight/ln_bias: [128] f32.  Output: scalar f32.

Per-core plan (Bc = 8 batch elements), bf16 data path:
  - Host packs per-core row tables: the pos/neg z gathers (with collided
    negatives zeroed, reproducing mask_from_map) land as zg [128L, 16seg*128]
    bf16, the c gather lands pre-transposed as cgt [128c, 8b*128L] bf16, and
    wpk = [W'^T | I].  Device reads only ~832KB contiguous per core.
  - LN stats via per-segment bn_stats (DVE); even/odd recombine and the
    Quake-rsqrt (2 Newton iters, eps and the /128 folded into the magic
    constants) run on the otherwise-idle gpsimd, so ACT only ever needs the
    exp_and_others table set (one load, prefetched by a dummy at t~0).
  - Normalize with the GroupNorm idiom: one tensor_scalar per segment with
    per-partition (mu, rstd) vector scalars, bf16 out.
  - PE transposes zln; MT computed as pm2[i,j] = zt^T E per batch half, so
    den = sum_i exp and num = diag(exp) are accumulating PE matmuls against
    a ones column (identity-masked product for the diagonal, on gpsimd).
  - Device outputs num/den [128, 16] f32; host does log(num/den + 1e-3) and
    the mean in float64.

ln_weight folds into W on the host; ln_bias cancels in the softmax.  No
max-subtraction needed: |logits| < ~70.
"""

import numpy as np

SEQ, B, L, ZD, CD = 2048, 64, 128, 128, 128
NCORES = 8
BC = B // NCORES  # 8
NSEG = 2 * BC  # 16 LN segments per core (interleaved pos/neg)
LN_EPS = 1e-5
SEM_STOP = 168  # min workable; exit sem sweep is ~65ns/sem

_cached = None


def _build_program():
    import concourse.bacc as bacc
    import concourse.tile as tile
    from concourse import bass as _bass
    from concourse import mybir

    # Fewer kernel semaphores -> shorter fixed entry/exit semaphore sweeps.
    orig_range = _bass.get_kernel_semaphore_range
    _bass.get_kernel_semaphore_range = lambda: range(
        orig_range().start, SEM_STOP
    )

    f32 = mybir.dt.float32
    bf16 = mybir.dt.bfloat16
    i32 = mybir.dt.int32
    AF = mybir.ActivationFunctionType
    ALU = mybir.AluOpType

    try:
        nc = bacc.Bacc(
            "TRN2",
            target_bir_lowering=False,
            debug=False,
            enable_asserts=True,
            num_devices=NCORES,
        )

        zg0_d = nc.dram_tensor("zg0", [128, 8 * ZD], bf16, kind="ExternalInput")
        zg1_d = nc.dram_tensor("zg1", [128, 8 * ZD], bf16, kind="ExternalInput")
        cgt_d = nc.dram_tensor("cgt", [128, BC * L], bf16, kind="ExternalInput")
        wpk_d = nc.dram_tensor("wpk", [128, 256], bf16, kind="ExternalInput")
        out_d = nc.dram_tensor("out", [NSEG, 128], f32, kind="ExternalOutput")

        with tile.TileContext(nc) as tc:
            with (
                tc.tile_pool(name="singles", bufs=1) as singles,
                tc.tile_pool(name="szt", bufs=3) as szt,
                tc.tile_pool(name="sexp", bufs=2) as sexp,
                tc.tile_pool(name="ppe", bufs=2, space="PSUM") as ppe,
                tc.tile_pool(name="ppzt", bufs=2, space="PSUM") as ppzt,
                tc.tile_pool(name="ppmt", bufs=2, space="PSUM") as ppmt,
                tc.tile_pool(name="ppout", bufs=1, space="PSUM") as ppout,
            ):
                # ---- input DMAs: zg quartered so stats start early; the
                # first two quarters go out on the scalar HWDGE ring (in
                # parallel with sync's) before the ACT table-load dummy.
                junk = singles.tile([128, 1], f32)
                nc.vector.memset(junk[:], 1.0)
                zg = singles.tile([128, NSEG * ZD], bf16)
                for q in range(4):
                    srct = (zg0_d, zg1_d)[q // 2]
                    half = (q % 2) * 512
                    eng = nc.scalar if q < 2 else nc.sync
                    eng.dma_start(
                        out=zg[:, q * 512 : (q + 1) * 512],
                        in_=srct.ap()[:, half : half + 512],
                    )
                # exp_and_others is the ONLY table set the kernel ever needs
                # (Exp + Copy + Identity); fetched before data lands.
                nc.scalar.activation(junk[:], junk[:], AF.Exp)
                wpk = singles.tile([128, 256], bf16)
                nc.sync.dma_start(wpk[:], wpk_d.ap())
                cgt = singles.tile([128, BC * L], bf16)
                nc.sync.dma_start(cgt[:], cgt_d.ap())
                wt = wpk[:, 0:128]
                identb = wpk[:, 128:256]

                # ---- c-side: E[z, b*L+j] = sum_c W'[z,c] c_t[j,b,c] ----
                e_sb = singles.tile([128, BC * L], bf16)
                for g in range(2):
                    pe = ppe.tile([128, 512], f32, tag="pe")
                    nc.tensor.matmul(
                        out=pe[:],
                        lhsT=wt,
                        rhs=cgt[:, g * 512 : (g + 1) * 512],
                        start=True,
                        stop=True,
                    )
                    nc.scalar.copy(e_sb[:, g * 512 : (g + 1) * 512], pe[:])

                # ---- LN stats: bn_stats per segment (HW: out = 6/part) ----
                st = singles.tile([128, NSEG, 6], f32)
                for s in range(NSEG):
                    nc.vector.bn_stats(
                        out=st[:, s, :], in_=zg[:, s * ZD : (s + 1) * ZD]
                    )

                # ---- even/odd recombine on DVE (it idles here anyway);
                # rstd = exp(-0.5*ln(128*(var+eps)/128)) on ACT, one op each.
                # bn_stats gives (cnt, mean, cnt*var) of even and odd elems.
                # mu = (me+mo)/2;  128*(var+eps) = (sve+svo) + 32*d^2 + 128eps
                mu = singles.tile([128, NSEG], f32)
                dt_ = singles.tile([128, NSEG], f32)
                dd = singles.tile([128, NSEG], f32)
                vt = singles.tile([128, NSEG], f32)
                sv = singles.tile([128, NSEG], f32)
                vvx = singles.tile([128, NSEG], f32)
                lnv = singles.tile([128, NSEG], f32)
                rstd = singles.tile([128, NSEG], f32)

                def stv(k):
                    return st[:, :, k : k + 1]

                u = lambda t: t[:].unsqueeze(-1)
                nc.vector.tensor_tensor(
                    out=u(dt_), in0=stv(1), in1=stv(4), op=ALU.subtract
                )
                nc.vector.tensor_tensor(
                    out=u(dd), in0=u(dt_), in1=u(dt_), op=ALU.mult
                )
                nc.vector.tensor_scalar(
                    out=dd[:], in0=dd[:], scalar1=32.0,
                    scalar2=128.0 * LN_EPS, op0=ALU.mult, op1=ALU.add,
                )
                nc.vector.tensor_tensor(
                    out=u(vt), in0=stv(2), in1=stv(5), op=ALU.add
                )
                nc.vector.tensor_tensor(
                    out=vvx[:], in0=vt[:], in1=dd[:], op=ALU.add
                )
                nc.vector.tensor_tensor(
                    out=u(sv), in0=stv(1), in1=stv(4), op=ALU.add
                )
                nc.vector.tensor_scalar(
                    out=mu[:], in0=sv[:], scalar1=0.5, scalar2=None,
                    op0=ALU.mult,
                )
                negmu = singles.tile([128, NSEG], f32)
                negms = singles.tile([128, NSEG], f32)
                nc.vector.tensor_scalar(
                    out=negmu[:], in0=sv[:], scalar1=-0.5, scalar2=None,
                    op0=ALU.mult,
                )
                # Quake rsqrt of vv = vvx/128 (the /128 folds into the magic
                # constant and the Newton -0.5 scale); one Newton iteration
                # brings the ~3e-3 seed error to ~2e-5.
                nc.vector.tensor_scalar(
                    out=rstd[:].bitcast(i32), in0=vvx[:].bitcast(i32),
                    scalar1=1, scalar2=None, op0=ALU.arith_shift_right,
                )
                nc.vector.tensor_scalar(
                    out=rstd[:].bitcast(i32), in0=rstd[:].bitcast(i32),
                    scalar1=-1, scalar2=0x5F3759DF + 0x01C00000,
                    op0=ALU.mult, op1=ALU.add,
                )
                nc.vector.tensor_tensor(
                    out=lnv[:], in0=rstd[:], in1=rstd[:], op=ALU.mult
                )
                nc.vector.tensor_tensor(
                    out=lnv[:], in0=lnv[:], in1=vvx[:], op=ALU.mult
                )
                nc.vector.tensor_scalar(
                    out=lnv[:], in0=lnv[:], scalar1=-0.5 / 128.0, scalar2=1.5,
                    op0=ALU.mult, op1=ALU.add,
                )
                nc.vector.tensor_tensor(
                    out=rstd[:], in0=rstd[:], in1=lnv[:], op=ALU.mult
                )
                nc.vector.tensor_tensor(
                    out=negms[:], in0=negmu[:], in1=rstd[:], op=ALU.mult
                )

                # ---- normalize: one tensor_scalar per segment ----
                zln = singles.tile([128, NSEG * ZD], bf16)

                def norm_seg(s):
                    if 8 <= s < 12:
                        # ACT Identity: zg*rstd + (-mu*rstd), per-partition APs
                        nc.scalar.activation(
                            out=zln[:, s * ZD : (s + 1) * ZD],
                            in_=zg[:, s * ZD : (s + 1) * ZD],
                            func=AF.Identity,
                            bias=negms[:, s : s + 1],
                            scale=rstd[:, s : s + 1],
                        )
                    else:
                        nc.vector.tensor_scalar(
                            out=zln[:, s * ZD : (s + 1) * ZD],
                            in0=zg[:, s * ZD : (s + 1) * ZD],
                            scalar1=mu[:, s : s + 1],
                            scalar2=rstd[:, s : s + 1],
                            op0=ALU.subtract,
                            op1=ALU.mult,
                        )

                for s in range(8):
                    norm_seg(s)

                # ---- per pair p: transposes; MT in [i-part, j-free] layout.
                # num/den land as [1,128] PSUM rows (lhsT=ones, rhs=expm) in
                # a [16,128] tile: rows 0-7 num, 8-15 den -> 16-desc out DMA.
                outv = singles.tile([128, NSEG], bf16)
                outvT = singles.tile([NSEG, 128], f32)
                onescol = singles.tile([128, 1], bf16)
                nc.vector.memset(onescol[:], 1.0)
                outp = ppout.tile([128, NSEG], f32, tag="outp")

                def pair(p):
                    pzt = ppzt.tile([128, 512], bf16, tag="pzt")
                    for k in range(4):
                        s = 4 * p + k
                        nc.tensor.transpose(
                            out=pzt[:, k * 128 : (k + 1) * 128],
                            in_=zln[:, s * ZD : (s + 1) * ZD],
                            identity=identb,
                        )
                    zt = szt.tile([128, 512], bf16, tag="zt")
                    if p == 1:
                        nc.scalar.copy(zt[:], pzt[:])
                    else:
                        nc.vector.tensor_copy(zt[:], pzt[:])
                    # pm2 cols [(2k+h)*128]: batch 2p+k, half h (pos/neg i)
                    pm2 = ppmt.tile([128, 512], f32, tag="pm2")
                    for q in range(4):
                        b = 2 * p + q // 2
                        nc.tensor.matmul(
                            out=pm2[:, q * 128 : (q + 1) * 128],
                            lhsT=zt[:, q * 128 : (q + 1) * 128],
                            rhs=e_sb[:, b * L : (b + 1) * L],
                            start=True,
                            stop=True,
                        )
                    expm = sexp.tile([128, 512], bf16, tag="expm")
                    nc.scalar.activation(expm[:], pm2[:], AF.Exp)
                    for k in range(2):
                        b = 2 * p + k
                        # den[j,b] = sum_i expm[i, j]: accumulate both halves
                        nc.tensor.matmul(
                            out=outp[:, BC + b : BC + b + 1],
                            lhsT=expm[:, 2 * k * 128 : (2 * k + 1) * 128],
                            rhs=onescol[:],
                            start=True,
                            stop=False,
                        )
                        nc.tensor.matmul(
                            out=outp[:, BC + b : BC + b + 1],
                            lhsT=expm[:, (2 * k + 1) * 128 : (2 * k + 2) * 128],
                            rhs=onescol[:],
                            start=False,
                            stop=True,
                        )
                        # num[b-row, j] = expm[j,j] of pos half (identity mask)
                        # gpsimd for early pairs; DVE (free by then, lower
                        # latency) for the last pair's output tail.
                        prod = szt.tile([128, 128], bf16, tag="prod")
                        eng = nc.vector if p == 3 else nc.gpsimd
                        eng.tensor_tensor(
                            out=prod[:],
                            in0=expm[:, 2 * k * 128 : (2 * k + 1) * 128],
                            in1=identb,
                            op=ALU.mult,
                        )
                        nc.tensor.matmul(
                            out=outp[:, b : b + 1],
                            lhsT=prod[:],
                            rhs=onescol[:],
                            start=True,
                            stop=True,
                        )

                pair(0)
                for s in range(8, NSEG):
                    norm_seg(s)
                for p in range(1, 4):
                    pair(p)
                # transpose [128,16]->[16,128] so the out DMA is 16 fat
                # descriptors instead of 128 64B ones (fast completion).
                nc.vector.tensor_copy(outv[:], outp[:])
                pot = ppout.tile([NSEG, 128], bf16, tag="pot")
                nc.tensor.transpose(out=pot[:], in_=outv[:], identity=identb)
                nc.vector.tensor_copy(outvT[:], pot[:])
                nc.sync.dma_start(out_d.ap(), outvT[:])

        nc.compile()
        return nc
    finally:
        _bass.get_kernel_semaphore_range = orig_range


def _prep_in_maps(z, c, mask, neg_map, W, ln_weight):
    import ml_dtypes

    bf = ml_dtypes.bfloat16
    z = np.asarray(z, dtype=np.float32)
    c = np.asarray(c, dtype=np.float32)
    mask = np.asarray(mask).astype(np.int64)
    neg_map = np.asarray(neg_map).astype(np.int64)
    W = np.asarray(W, dtype=np.float32)
    ln_weight = np.asarray(ln_weight, dtype=np.float32)

    wt = (ln_weight[:, None] * W).T  # wt[c, z] = W'[z, c]
    wpk = np.ascontiguousarray(
        np.concatenate([wt, np.eye(128, dtype=np.float32)], axis=1)
    ).astype(bf)
    boff = np.arange(BC)[None, :]
    in_maps = []
    for i in range(NCORES):
        bsl = slice(i * BC, (i + 1) * BC)
        m = mask[:, bsl]  # [L, BC]
        n = neg_map[:, bsl]
        zb = z[:, bsl, :]
        cb = c[:, bsl, :]
        zpos = zb[m, boff, :]  # [L, BC, ZD]
        zneg = zb[n, boff, :]
        hit = (n[:, None, :] == m[None, :, :]).any(axis=1)  # [L, BC]
        zneg = np.where(hit[:, :, None], np.float32(0.0), zneg)
        zga = np.empty((L, NSEG, ZD), dtype=np.float32)
        zga[:, 0::2, :] = zpos
        zga[:, 1::2, :] = zneg
        zga16 = zga.astype(bf)
        zg0 = np.ascontiguousarray(zga16[:, 0:8, :].reshape(L, 8 * ZD))
        zg1 = np.ascontiguousarray(zga16[:, 8:16, :].reshape(L, 8 * ZD))
        cpos = cb[m, boff, :]  # [L(j), BC, CD]
        cgt = np.ascontiguousarray(
            cpos.transpose(2, 1, 0).reshape(CD, BC * L)
        ).astype(bf)
        in_maps.append({"zg0": zg0, "zg1": zg1, "cgt": cgt, "wpk": wpk})
    return in_maps


def _combine(results):
    total = np.float64(0.0)
    for r in results:
        o = np.asarray(r["out"], dtype=np.float64)  # [16, 128]: num rows, den rows
        num, den = o[0:BC, :], o[BC : 2 * BC, :]
        total += np.log(num / den + 1e-3).sum()
    return np.float32(-(total / (L * B)))


def kernel(z, c, mask, neg_map, W, ln_weight, ln_bias):
    from concourse import bass_utils

    global _cached
    if _cached is None:
        _cached = _build_program()
    nc = _cached

    in_maps = _prep_in_maps(z, c, mask, neg_map, W, ln_weight)
    res = bass_utils.run_bass_kernel_spmd(
        nc, in_maps, core_ids=list(range(NCORES))
    )
    return _combine(res.results)
